# revision 5
# baseline (speedup 1.0000x reference)
"""v5: all-fp32r PE compute; mixed-precision HBM streams; pre-shuffled layouts.

Measured on HW here: fp32r N=512 matmuls sustain ~222ns vs bf16's 259ns, the
compiler forbids mixing 32/16-bit matmul operands, GpSimd casts are ~7us
(unusable), Scalar casts ~2us, and 1KB DMA descriptors run at ~22GB/s/engine
(descriptor-bound). Hence:
 - every input is PRE-SHUFFLED on the host to the SBUF tile layout
   [BLOC, 128, KB, C] so each DMA is 4-8KB contiguous per partition,
 - u and g stream as fp32r (they feed 3 of the 6 matmul operand slots),
 - ut/gt/a/lamt stream bf16 and are upcast on Scalar (lamt, ut) and DVE
   (gt, a) one batch ahead of use,
 - all 5 matmuls + the S^T transpose set run fp32r; outputs are written bf16
   (host upcasts) in shuffled layout (host un-shuffles).
"""
import numpy as np
import ml_dtypes

import concourse.bass as bass
import concourse.mybir as mybir
import concourse.tile as tile
from concourse import bacc
from concourse.bass_utils import run_bass_kernel_spmd
from concourse.masks import make_identity

F32 = mybir.dt.float32
F32R = mybir.dt.float32r
BF16 = mybir.dt.bfloat16
AOP = mybir.AluOpType

B, N, P = 64, 512, 512
NCORES = 8
BLOC = B // NCORES
KB = 4
CH = 4


def _build_nc():
    nc = bacc.Bacc("TRN2", target_bir_lowering=False, debug=False,
                   num_devices=NCORES)

    # all dram tensors pre-shuffled to [BLOC, 128, KB, C] (SBUF tile layout)
    d_u = nc.declare_dram_parameter("u", [BLOC, 128, KB, P], F32R, isOutput=False)
    d_g = nc.declare_dram_parameter("g", [BLOC, 128, KB, P], F32R, isOutput=False)
    d_ut = nc.declare_dram_parameter("ut", [BLOC, 128, KB, N], BF16, isOutput=False)
    d_gt = nc.declare_dram_parameter("gt", [BLOC, 128, KB, N], BF16, isOutput=False)
    d_a = nc.declare_dram_parameter("a", [BLOC, 128, KB, P], BF16, isOutput=False)
    d_om = nc.declare_dram_parameter("om", [BLOC, 128, KB, P], BF16, isOutput=False)
    d_lamt = nc.declare_dram_parameter("lamt", [BLOC, 128, KB, N], BF16, isOutput=False)
    d_u0 = nc.declare_dram_parameter("u0", [128, KB, P], BF16, isOutput=False)
    d_g0 = nc.declare_dram_parameter("g0", [128, KB, P], BF16, isOutput=False)
    d_du = nc.declare_dram_parameter("du", [BLOC, 128, KB, P], BF16, isOutput=True)
    d_dlam = nc.declare_dram_parameter("dlam", [BLOC, 128, KB, P], BF16, isOutput=True)

    with tile.TileContext(nc) as tc:
        with (
            tc.tile_pool(name="const", bufs=1) as constp,
            tc.tile_pool(name="ins", bufs=2) as insp,
            tc.tile_pool(name="mid", bufs=1) as midp,
            tc.tile_pool(name="outs", bufs=2) as outsp,
            tc.tile_pool(name="psum", bufs=8, space="PSUM") as psum,
        ):
            # HAM warm-up gated only on two fast DVE memsets
            warm_ps = psum.tile([128, 512], F32, tag="ps")
            wlhs = constp.tile([128, 128], F32R)
            wsrc = constp.tile([128, 512], F32R)
            nc.vector.memset(wlhs[:].bitcast(F32), 0.0)
            nc.vector.memset(wsrc[:].bitcast(F32), 0.0)
            for i in range(6):
                nc.tensor.matmul(warm_ps[:], wlhs[:], wsrc[:],
                                 start=True, stop=True)

            ident = constp.tile([128, 128], F32)
            make_identity(nc, ident[:])
            identr = constp.tile([128, 128], F32R)
            nc.vector.tensor_copy(identr[:], ident[:])

            tiles = {}

            def stage_dma(b):
                """DMA batch b's inputs."""
                u_sb = insp.tile([128, KB, P], F32R, tag="u")
                g_sb = insp.tile([128, KB, P], F32R, tag="g")
                ut_st = insp.tile([128, KB, N], BF16, tag="ut")
                gt_st = insp.tile([128, KB, N], BF16, tag="gt")
                a_st = insp.tile([128, KB, P], BF16, tag="a")
                om_sb = insp.tile([128, KB, P], BF16, tag="om")
                lamt_st = insp.tile([128, KB, N], BF16, tag="lamt")
                if b == 0:
                    u_sb = insp.tile([128, KB, P], BF16, tag="u0", bufs=1)
                    g_sb = insp.tile([128, KB, P], BF16, tag="g0", bufs=1)
                    nc.sync.dma_start(u_sb[:], d_u0[:])
                    nc.scalar.dma_start(g_sb[:], d_g0[:])
                else:
                    nc.sync.dma_start(u_sb[:], d_u[b])
                    nc.scalar.dma_start(g_sb[:], d_g[b])
                nc.scalar.dma_start(lamt_st[:], d_lamt[b])
                nc.sync.dma_start(gt_st[:], d_gt[b])
                nc.sync.dma_start(om_sb[:], d_om[b])
                nc.scalar.dma_start(ut_st[:], d_ut[b])
                nc.sync.dma_start(a_st[:], d_a[b])
                tiles[b] = [u_sb, g_sb, ut_st, gt_st, a_st, om_sb, lamt_st]

            def stage_cast_scalar(b):
                """Scalar upcasts for batch b (behind the prior S copies)."""
                t = tiles[b]
                lamtf = insp.tile([128, KB, N], F32R, tag="lamtf")
                utf = insp.tile([128, KB, N], F32R, tag="utf")
                nc.scalar.copy(lamtf[:], t[6][:])
                nc.scalar.copy(utf[:], t[2][:])
                tiles[b] = t[:7] + [utf, lamtf]

            def stage_cast_dve(b):
                """DVE upcasts for batch b. Emitted after the prior batch's
                coup copies: putting them earlier head-of-line-blocks the DVE
                FIFO on batch b's late-landing DMAs, stalling the prior
                batch's du adds and coup copies (and with them lam@A)."""
                t = tiles[b]
                gtf = insp.tile([128, KB, N], F32R, tag="gtf")
                af = insp.tile([128, KB, P], F32R, tag="af")
                nc.vector.tensor_copy(gtf[:], t[3][:])
                nc.vector.tensor_copy(af[:], t[4][:])
                tiles[b] = t + [gtf, af]

            stage_dma(0)
            for b in range(BLOC):
                if b + 1 < BLOC:
                    stage_dma(b + 1)
                t = tiles.pop(b)
                u_sb, g_sb, om_sb = t[0], t[1], t[5]
                if b == 0:
                    # batch 0 runs fully in bf16 off the staging tiles
                    # (cold-clock-immune, no cast gates)
                    utf, gtf, af, lamtf = t[2], t[3], t[4], t[6]
                else:
                    utf, lamtf, gtf, af = t[7], t[8], t[9], t[10]

                # ---- M1: UTG = u^T G ; W = Omega - UTG (DVE) ----
                wdt = BF16 if b == 0 else F32R
                w_sb = midp.tile([128, KB, P], wdt, tag="w0" if b == 0 else "w",
                                 bufs=1 if b == 0 else 2)
                for r in range(CH):
                    utg = psum.tile([128, P], F32, tag="ps")
                    for k in range(KB):
                        nc.tensor.matmul(utg[:], u_sb[:, k, r * 128:(r + 1) * 128],
                                         g_sb[:, k, :], start=(k == 0), stop=(k == KB - 1))
                    nc.vector.tensor_tensor(w_sb[:, r, :], om_sb[:, r, :],
                                            utg[:], AOP.subtract)

                # ---- M5: S = lam @ G^T (group left open for S^T accumulation) ----
                s_ps = []
                s_sb = midp.tile([128, KB, N], F32R, tag="s")
                for r in range(CH):
                    ps = psum.tile([128, N], F32, tag="ps")
                    for k in range(KB):
                        nc.tensor.matmul(ps[:], lamtf[:, k, r * 128:(r + 1) * 128],
                                         gtf[:, k, :], start=(k == 0), stop=False)
                    nc.scalar.copy(s_sb[:, r, :], ps[:])
                    s_ps.append(ps)

                # scalar upcasts for the next batch behind this batch's S copies
                if b + 1 < BLOC:
                    stage_cast_scalar(b + 1)

                # ---- M23: du = u @ W + G ----
                du_sb = outsp.tile([128, KB, P], BF16, tag="du")
                for r in range(CH):
                    ps = psum.tile([128, P], F32, tag="ps")
                    for k in range(KB):
                        nc.tensor.matmul(ps[:], utf[:, k, r * 128:(r + 1) * 128],
                                         w_sb[:, k, :], start=(k == 0), stop=(k == KB - 1))
                    nc.vector.tensor_tensor(du_sb[:, r, :], ps[:],
                                            g_sb[:, r, :], AOP.add)
                nc.sync.dma_start(d_du[b], du_sb[:])

                # ---- S^T accumulated into S's PSUM -> C = S + S^T ----
                coup_sb = midp.tile([128, KB, N], BF16 if b == 0 else F32R,
                                    tag="coup0" if b == 0 else "coup")
                for r in range(CH):
                    for c in range(KB):
                        nc.tensor.matmul(
                            s_ps[r][:, c * 128:(c + 1) * 128].bitcast(F32R),
                            s_sb[:, c, r * 128:(r + 1) * 128],
                            identr[:],
                            is_transpose=True,
                            start=False, stop=(c == KB - 1),
                        )
                for r in range(CH):
                    nc.vector.tensor_copy(coup_sb[:, r, :], s_ps[r][:])
                # DVE upcasts for the next batch only after the coup copies
                if b + 1 < BLOC:
                    stage_cast_dve(b + 1)

                # ---- M4+M7: dlam = lam @ A + C @ u ----
                dlam_sb = outsp.tile([128, KB, P], BF16, tag="dlam")
                dlam_ps = []
                for r in range(CH):
                    ps = psum.tile([128, P], F32, tag="ps")
                    for k in range(KB):
                        nc.tensor.matmul(ps[:], lamtf[:, k, r * 128:(r + 1) * 128],
                                         af[:, k, :], start=(k == 0), stop=False)
                    dlam_ps.append(ps)
                for r in range(CH):
                    ps = dlam_ps[r]
                    for k in range(KB):
                        nc.tensor.matmul(ps[:], coup_sb[:, k, r * 128:(r + 1) * 128],
                                         u_sb[:, k, :], start=False, stop=(k == KB - 1))
                    if b == BLOC - 1:
                        if r % 2 == 0:
                            nc.vector.tensor_copy(dlam_sb[:, r, :], ps[:])
                            nc.sync.dma_start(d_dlam[b][:, r], dlam_sb[:, r, :])
                        else:
                            nc.scalar.copy(dlam_sb[:, r, :], ps[:])
                            nc.scalar.dma_start(d_dlam[b][:, r], dlam_sb[:, r, :])
                    else:
                        nc.scalar.copy(dlam_sb[:, r, :], ps[:])
                if b < BLOC - 1:
                    nc.scalar.dma_start(d_dlam[b], dlam_sb[:])

    nc.compile()
    return nc


_NC = None


def _shuf(x):
    """[BLOC, R, C] -> [BLOC, 128, R//128, C] (SBUF tile layout, contiguous)"""
    bl, rr, cc = x.shape
    return np.ascontiguousarray(
        x.reshape(bl, rr // 128, 128, cc).transpose(0, 2, 1, 3))


def _unshuf(y):
    """[BLOC, 128, KB, C] -> [BLOC, 128*KB, C]"""
    bl, p, kb, cc = y.shape
    return y.transpose(0, 2, 1, 3).reshape(bl, p * kb, cc)


def _make_in_maps(u, lam, A, G):
    bf = ml_dtypes.bfloat16
    u = np.ascontiguousarray(u, dtype=np.float32)
    lam = np.ascontiguousarray(lam, dtype=np.float32)
    A = np.ascontiguousarray(A, dtype=np.float32)
    G = np.ascontiguousarray(G, dtype=np.float32)

    ub = _shuf(u)                                            # f32r
    gb = _shuf(G)                                            # f32r
    ub0 = ub.astype(bf)                                      # batch-0 bf16
    gb0 = gb.astype(bf)
    utb = _shuf(np.swapaxes(u, 1, 2)).astype(bf)
    gtb = _shuf(np.swapaxes(G, 1, 2)).astype(bf)
    ab = _shuf(A).astype(bf)
    omb = _shuf(0.5 * (A - np.swapaxes(A, 1, 2))).astype(bf)
    lamtb = _shuf(np.swapaxes(lam, 1, 2)).astype(bf)

    in_maps = []
    for c in range(NCORES):
        sl = slice(c * BLOC, (c + 1) * BLOC)
        in_maps.append({
            "u": ub[sl], "g": gb[sl], "ut": utb[sl], "gt": gtb[sl],
            "a": ab[sl], "om": omb[sl], "lamt": lamtb[sl],
            "u0": ub0[sl.start], "g0": gb0[sl.start],
        })
    return in_maps


def kernel(u, lam, A, G, t=None, **_ignored):
    global _NC
    if _NC is None:
        _NC = _build_nc()
    nc = _NC

    in_maps = _make_in_maps(u, lam, A, G)
    res = run_bass_kernel_spmd(nc, in_maps, list(range(NCORES)))
    du = np.concatenate([_unshuf(res.results[c]["du"]) for c in range(NCORES)],
                        axis=0).astype(np.float32)
    dlam = np.concatenate([_unshuf(res.results[c]["dlam"]) for c in range(NCORES)],
                          axis=0).astype(np.float32)
    return du, dlam
